# revision 9
# baseline (speedup 1.0000x reference)
"""Trainium2 Bass kernel for an 8-block vision transformer (ArlowVisionTransformer).

Megatron-style tensor parallelism across the 8 NeuronCores of one chip: each
core owns 2 of 16 attention heads (qkv column-sharded), a 640-column slice of
fc1/m0 and the matching row slices of proj/fc2/m2; row-parallel partials are
AllReduced (bf16 bounce).

The residual stream is kept TRANSPOSED (features on SBUF partitions, tokens on
the free axis) so every linear layer consumes it directly with weight slices
stationary — no activation transposes.  LayerNorm uses "late normalization":
per-token stats come from ones-matmuls on TensorE, mean subtraction is folded
into each consumer matmul as a rank-1 accumulation, and the rstd scale is
applied to the (smaller) outputs.  LN affine params and the attention scale
are folded into weights on the host.  RoPE's rotate-half is an 80x80 matmul;
softmax denominators come from a ones-column appended to V.  Tokens run in 2
chunks so per-chunk AllReduces overlap the other chunk's compute.
"""

import sys

sys.path.insert(0, "/opt/trn_rl_repo")

import numpy as np
import ml_dtypes

import concourse.bass as bass
import concourse.bacc as bacc
import concourse.tile as tile
import concourse.mybir as mybir
from concourse import bass_utils

F32 = mybir.dt.float32
BF16 = mybir.dt.bfloat16
ALU = mybir.AluOpType
ACTF = mybir.ActivationFunctionType

DEPTH = 8
D = 1280
HEADS = 16
HD = 80
MLPD = 5120
OUT_DIM = 3584
S = 1024
NCORE = 8
HLOC = HEADS // NCORE
CH = 2
CS = S // CH
NDT = D // 128
NST = CS // 128
NF1 = 5
EPS = 1e-6
QKC = 512
FLOC = MLPD // NCORE
NPT = 1176
HB = 100                # per-head v block: 80 v + 16 zeros + 1 one + 3 pad
VW = 2 * HB
G = S // 4
GC = G // CH            # merged-token groups per chunk

COMM_DT = BF16
WGT_DT = BF16
ATT_DT = BF16

_NPW = ml_dtypes.bfloat16 if WGT_DT == BF16 else np.float32
_NPA = ml_dtypes.bfloat16 if ATT_DT == BF16 else np.float32


def _build(has_qkv_b):
    nc = bacc.Bacc("TRN2", target_bir_lowering=False, debug=False,
                   enable_asserts=False, num_devices=NCORE)

    def din(name, shape, dt=F32):
        return nc.dram_tensor(name, list(shape), dt, kind="ExternalInput")

    t = {}
    t["xT"] = din("xT", (D, S), ATT_DT)
    t["convT"] = din("convT", (D, 2, 128), ATT_DT)
    t["cosT"] = din("cosT", (HD, S), ATT_DT)
    t["sinT"] = din("sinT", (HD, S), ATT_DT)
    t["rotM"] = din("rotM", (HD, HD), ATT_DT)
    t["qkT"] = din("qkT", (DEPTH, D, QKC), WGT_DT)
    t["qkcs"] = din("qkcs", (DEPTH, QKC), WGT_DT)
    t["vT"] = din("vT", (DEPTH, D, HLOC * HD), WGT_DT)
    t["vcs"] = din("vcs", (DEPTH, HLOC * HD), WGT_DT)
    t["pT"] = din("pT", (DEPTH, HLOC, HD, D), ATT_DT)
    t["p_b"] = din("p_b", (DEPTH, D))
    t["f1T"] = din("f1T", (DEPTH, D, FLOC), WGT_DT)
    t["f1cs"] = din("f1cs", (DEPTH, FLOC), WGT_DT)
    t["f1b"] = din("f1b", (DEPTH, FLOC))
    t["f2T"] = din("f2T", (DEPTH, FLOC, D), WGT_DT)
    t["f2b"] = din("f2b", (DEPTH, D))
    t["m0T"] = din("m0T", (MLPD, FLOC), WGT_DT)
    t["m0b"] = din("m0b", (FLOC,))
    t["m2T"] = din("m2T", (FLOC, OUT_DIM), WGT_DT)
    t["m2b"] = din("m2b", (OUT_DIM,))
    if has_qkv_b:
        t["qkb"] = din("qkb", (DEPTH, QKC))
        t["vb"] = din("vb", (DEPTH, HLOC * HD))
    t["out"] = nc.dram_tensor("out", [OUT_DIM, G], F32, kind="ExternalOutput")

    with tile.TileContext(nc) as tc:
        with (
            tc.tile_pool(name="const", bufs=1) as pc,
            tc.tile_pool(name="resid", bufs=1) as ph,
            tc.tile_pool(name="wgt", bufs=1) as pw,
            tc.tile_pool(name="big", bufs=2) as pbig,
            tc.tile_pool(name="qkv", bufs=1) as pqk,
            tc.tile_pool(name="stg", bufs=3) as ps,
            tc.tile_pool(name="st2", bufs=2) as ps2,
            tc.tile_pool(name="bias", bufs=2) as pb,
            tc.tile_pool(name="pmm", bufs=3, space="PSUM") as pmm,
            tc.tile_pool(name="prow", bufs=1, space="PSUM") as prow,
            tc.tile_pool(name="pops", bufs=2, space="PSUM") as pops,
            tc.tile_pool(name="dram", bufs=2, space="DRAM") as pd,
        ):
            pools = dict(pc=pc, ph=ph, pw=pw, pbig=pbig, pqk=pqk, ps=ps,
                         ps2=ps2, pb=pb, pmm=pmm, prow=prow, pops=pops, pd=pd)
            _body(nc, tc, t, pools, has_qkv_b)
    nc.compile()
    return nc


def _body(nc, tc, t, p, has_qkv_b):
    pc, ph, pw, pbig, pqk = p["pc"], p["ph"], p["pw"], p["pbig"], p["pqk"]
    ps, ps2, pb, pmm, prow, pops, pd = (p["ps"], p["ps2"], p["pb"], p["pmm"],
                                        p["prow"], p["pops"], p["pd"])
    RG = [list(range(NCORE))]

    # ---- constants ----
    ones_col = pc.tile([128, 1], F32)
    nc.gpsimd.memset(ones_col[:], 1.0)
    ones_row = pc.tile([1, 128], F32)
    nc.gpsimd.memset(ones_row[:], 1.0)
    cos_sb = pc.tile([HD, S], ATT_DT)
    nc.sync.dma_start(cos_sb[:], t["cosT"][:])
    sin_sb = pc.tile([HD, S], ATT_DT)
    nc.sync.dma_start(sin_sb[:], t["sinT"][:])
    rot_sb = pc.tile([HD, HD], ATT_DT)
    nc.sync.dma_start(rot_sb[:], t["rotM"][:])
    eps_sb = pc.tile([128, 1], F32)
    nc.gpsimd.memset(eps_sb[:], EPS)

    hT = ph.tile([128, NDT * S], F32)

    # ---- patchify ----
    cw = pw.tile([128, NDT, 2, 128], ATT_DT, tag="cw")
    nc.sync.dma_start(cw[:],
                      t["convT"][:].rearrange("(a p) m x -> p a m x", p=128))
    pf_in = pd.tile([D // NCORE, S], F32, tag="pfi")
    pf_out = pd.tile([D, S], F32, tag="pfo", addr_space="Shared")
    for nt in range(4):
        xin = pbig.tile([128, NDT, 256], ATT_DT, tag="f1a")
        nc.sync.dma_start(
            xin[:], t["xT"][:, nt * 256:(nt + 1) * 256]
            .rearrange("(a p) s -> p a s", p=128))
        for mt in range(2):
            mw_ = 128 if mt == 0 else 32
            acc = pmm.tile([128, 256], F32, tag="mm")
            for kt in range(NDT):
                nc.tensor.matmul(acc[:mw_, :], cw[:, kt, mt, :mw_],
                                 xin[:, kt, :],
                                 start=(kt == 0), stop=(kt == NDT - 1))
            st = ps.tile([128, 256], F32, tag="t512")
            nc.scalar.copy(st[:mw_, :], acc[:mw_, :])
            nc.sync.dma_start(
                pf_in[mt * 128:mt * 128 + mw_, nt * 256:(nt + 1) * 256],
                st[:mw_, :])
    nc.gpsimd.collective_compute(
        "AllGather", ALU.bypass, replica_groups=RG,
        ins=[pf_in[:].opt()], outs=[pf_out[:].opt()])
    nc.sync.dma_start(hT[:].rearrange("p (a s) -> p a s", a=NDT),
                      pf_out[:].rearrange("(a p) s -> p a s", p=128))

    # ---- helpers ----
    def ln_stats(c):
        sums = prow.tile([1, CS], F32, tag="srow")
        sq = prow.tile([1, CS], F32, tag="sqrow")
        for dt in range(NDT):
            sl = hT[:, dt * S + c * CS: dt * S + (c + 1) * CS]
            x2 = ps.tile([128, CS], F32, tag="t512")
            nc.scalar.square(x2[:], sl)
            nc.tensor.matmul(sums[:], ones_col[:], sl,
                             start=(dt == 0), stop=(dt == NDT - 1))
            nc.tensor.matmul(sq[:], ones_col[:], x2[:],
                             start=(dt == 0), stop=(dt == NDT - 1))
        negm = ps2.tile([1, CS], F32, tag="negm")
        nc.vector.tensor_scalar_mul(negm[:], sums[:], -1.0 / D)
        msq = ps2.tile([1, CS], F32, tag="msq")
        nc.vector.tensor_tensor(msq[:], negm[:], negm[:], op=ALU.mult)
        var = ps2.tile([1, CS], F32, tag="var")
        nc.vector.scalar_tensor_tensor(var[:], sq[:], 1.0 / D, msq[:],
                                       op0=ALU.mult, op1=ALU.subtract)
        sqv = ps2.tile([1, CS], F32, tag="sqv")
        nc.scalar.activation(sqv[:], var[:], ACTF.Sqrt, bias=eps_sb[0:1, :], scale=1.0)
        rstd = ps2.tile([1, CS], F32, tag="rstd")
        nc.vector.reciprocal(rstd[:], sqv[:])
        rbp = pmm.tile([128, CS], F32, tag="mm")
        nc.tensor.matmul(rbp[:], ones_row[:], rstd[:], start=True, stop=True)
        rbs = ps2.tile([128, CS], F32, tag="rbs")
        nc.scalar.copy(rbs[:], rbp[:])
        negm_w = negm
        if WGT_DT != F32:
            negm_w = ps2.tile([1, CS], WGT_DT, tag="negmw")
            nc.vector.tensor_copy(negm_w[:], negm[:])
        return negm, negm_w, rstd, sqv, rbs

    def residual_add(c, ar_dram):
        for dt in range(NDT):
            ars = ps.tile([128, CS], COMM_DT, tag="cst")
            nc.sync.dma_start(ars[:], ar_dram[dt * 128:(dt + 1) * 128, :])
            sl = hT[:, dt * S + c * CS: dt * S + (c + 1) * CS]
            nc.gpsimd.tensor_tensor(sl, ars[:], sl, op=ALU.add)

    ar2_out_prev = [None, None]

    # ---- transformer blocks ----
    for l in range(DEPTH):
        qk_sb = pw.tile([128, NDT, QKC], WGT_DT, tag="qkT")
        nc.sync.dma_start(qk_sb[:],
                          t["qkT"][l].rearrange("(a p) m -> p a m", p=128))
        qkcs_sb = pb.tile([1, QKC], WGT_DT, tag="qkcs")
        nc.sync.dma_start(qkcs_sb[:], t["qkcs"][l:l + 1, :])
        v_wsb = pw.tile([128, NDT, HLOC * HD], WGT_DT, tag="vT")
        nc.sync.dma_start(v_wsb[:],
                          t["vT"][l].rearrange("(a p) m -> p a m", p=128))
        vcs_sb = pb.tile([1, HLOC * HD], WGT_DT, tag="vcs")
        nc.sync.dma_start(vcs_sb[:], t["vcs"][l:l + 1, :])
        p_wsb = pw.tile([HD, HLOC, D], ATT_DT, tag="pT")
        nc.sync.dma_start(p_wsb[:], t["pT"][l].rearrange("j d m -> d j m"))
        pb_sb = pb.tile([128, NDT], F32, tag="pb")
        nc.sync.dma_start(pb_sb[:], t["p_b"][l].rearrange("(a p) -> p a", p=128))
        f1_sb = pw.tile([128, NDT, FLOC], WGT_DT, tag="f1T")
        nc.sync.dma_start(f1_sb[:],
                          t["f1T"][l].rearrange("(a p) m -> p a m", p=128))
        f1cs_sb = pb.tile([1, FLOC], WGT_DT, tag="f1cs")
        nc.sync.dma_start(f1cs_sb[:], t["f1cs"][l:l + 1, :])
        f1b_sb = pb.tile([128, NF1], F32, tag="f1b")
        nc.sync.dma_start(f1b_sb[:], t["f1b"][l].rearrange("(a p) -> p a", p=128))
        f2_sb = pw.tile([128, NF1, D], WGT_DT, tag="f2T")
        nc.sync.dma_start(f2_sb[:],
                          t["f2T"][l].rearrange("(a p) m -> p a m", p=128))
        f2b_sb = pb.tile([128, NDT], F32, tag="f2b")
        nc.sync.dma_start(f2b_sb[:], t["f2b"][l].rearrange("(a p) -> p a", p=128))
        if has_qkv_b:
            qkb_sb = pb.tile([1, QKC], F32, tag="qkb")
            nc.sync.dma_start(qkb_sb[:], t["qkb"][l:l + 1, :])
            vb_sb = pb.tile([1, HLOC * HD], F32, tag="vb")
            nc.sync.dma_start(vb_sb[:], t["vb"][l:l + 1, :])

        q_sb = pqk.tile([HD, HLOC * S], ATT_DT, tag="qsb")
        k_sb = pqk.tile([HD, HLOC * S], ATT_DT, tag="ksb")
        v_sb = pqk.tile([128, (S // 128) * VW], ATT_DT, tag="vsb")
        xbf = [None] * CH

        for c in range(CH):
            if ar2_out_prev[c] is not None:
                residual_add(c, ar2_out_prev[c])
                ar2_out_prev[c] = None

            negm, negm_w, rstd, sqv, rbs = ln_stats(c)
            xb = pbig.tile([128, NDT * CS], ATT_DT, tag="xbf")
            for dt in range(NDT):
                nc.vector.tensor_copy(xb[:, dt * CS:(dt + 1) * CS],
                                      hT[:, dt * S + c * CS: dt * S + (c + 1) * CS])
            xbf[c] = xb

            # q,k: psum = W^T x - colsum (x) mean [+ b (x) sqrtvar]
            for mt in range(4):
                mm = pmm.tile([128, CS], F32, tag="mm")
                for kt in range(NDT):
                    nc.tensor.matmul(mm[:], qk_sb[:, kt, mt * 128:(mt + 1) * 128],
                                     xb[:, kt * CS:(kt + 1) * CS],
                                     start=(kt == 0), stop=False)
                nc.tensor.matmul(mm[:], qkcs_sb[:, mt * 128:(mt + 1) * 128],
                                 negm_w[:], start=False, stop=not has_qkv_b)
                if has_qkv_b:
                    nc.tensor.matmul(mm[:], qkb_sb[:, mt * 128:(mt + 1) * 128],
                                     sqv[:], start=False, stop=True)
                t1 = ps.tile([HD, CS], ATT_DT, tag="tro")
                nc.vector.tensor_tensor(t1[:], mm[:HD, :], rbs[:HD, :],
                                        op=ALU.mult)
                rot = pmm.tile([HD, CS], F32, tag="mm")
                nc.tensor.matmul(rot[:], rot_sb[:], t1[:], start=True, stop=True)
                j, is_k = mt % 2, mt // 2
                dst = (k_sb if is_k else q_sb)[:, j * S + c * CS: j * S + (c + 1) * CS]
                t2 = ps.tile([HD, CS], ATT_DT, tag="tro")
                nc.vector.tensor_tensor(t2[:], t1[:],
                                        cos_sb[:, c * CS:(c + 1) * CS], op=ALU.mult)
                t3 = ps.tile([HD, CS], ATT_DT, tag="tro")
                nc.vector.tensor_tensor(t3[:], rot[:],
                                        sin_sb[:, c * CS:(c + 1) * CS], op=ALU.mult)
                nc.vector.tensor_tensor(dst, t2[:], t3[:], op=ALU.add)

            # v (normal layout) scaled by rstd per token
            rcol = ps.tile([128, NST], F32, tag="rcol")
            rtmp = pd.tile([CS], F32, tag="rtmp")
            nc.sync.dma_start(rtmp[:], rstd[0:1, :])
            nc.sync.dma_start(rcol[:],
                              rtmp[:].rearrange("(a p) -> p a", p=128))
            for st in range(NST):
                mm = pmm.tile([128, HLOC * HD], F32, tag="mm")
                for kt in range(NDT):
                    nc.tensor.matmul(
                        mm[:], xb[:, kt * CS + st * 128: kt * CS + (st + 1) * 128],
                        v_wsb[:, kt, :], start=(kt == 0), stop=False)
                nc.tensor.matmul(mm[:], negm_w[0:1, st * 128:(st + 1) * 128],
                                 vcs_sb[:], start=False, stop=not has_qkv_b)
                if has_qkv_b:
                    nc.tensor.matmul(mm[:], sqv[0:1, st * 128:(st + 1) * 128],
                                     vb_sb[:], start=False, stop=True)
                tt = c * NST + st
                for j in range(HLOC):
                    base = tt * VW + j * HB
                    nc.vector.tensor_scalar_mul(v_sb[:, base:base + HD],
                                                mm[:, j * HD:(j + 1) * HD],
                                                rcol[:, st:st + 1])
                    nc.gpsimd.memset(v_sb[:, base + HD:base + 96], 0.0)
                    nc.gpsimd.memset(v_sb[:, base + 96:base + 97], 1.0)

        # attention
        o_sb = pqk.tile([HD, HLOC * S], ATT_DT, tag="osb")
        for j in range(HLOC):
            for c in range(CH):
                att = pqk.tile([128, (S // 128) * CS], ATT_DT, tag="attT")
                for tt in range(S // 128):
                    mm = pmm.tile([128, CS], F32, tag="mm")
                    nc.tensor.matmul(
                        mm[:], k_sb[:, j * S + tt * 128:j * S + (tt + 1) * 128],
                        q_sb[:, j * S + c * CS:j * S + (c + 1) * CS],
                        start=True, stop=True)
                    nc.scalar.activation(att[:, tt * CS:(tt + 1) * CS], mm[:],
                                         ACTF.Exp)
                ops = pops.tile([97, CS], F32, tag="ops")
                for tt in range(S // 128):
                    base = tt * VW + j * HB
                    nc.tensor.matmul(ops[:], v_sb[:, base:base + 97],
                                     att[:, tt * CS:(tt + 1) * CS],
                                     start=(tt == 0), stop=(tt == S // 128 - 1))
                rec = ps2.tile([1, CS], F32, tag="rec")
                nc.vector.reciprocal(rec[:], ops[96:97, :])
                rb2 = pmm.tile([HD, CS], F32, tag="mm")
                nc.tensor.matmul(rb2[:], ones_row[0:1, :HD], rec[:],
                                 start=True, stop=True)
                oraw = ps.tile([HD, CS], ATT_DT, tag="tro")
                nc.scalar.copy(oraw[:], ops[:HD, :])
                nc.vector.tensor_tensor(
                    o_sb[:, j * S + c * CS: j * S + (c + 1) * CS],
                    oraw[:], rb2[:], op=ALU.mult)

        # proj partials -> AllReduce per chunk
        ar1_out = [None] * CH
        for c in range(CH):
            ar_in = pd.tile([D, CS], COMM_DT, tag="ar1i")
            ar1_out[c] = pd.tile([D, CS], COMM_DT, tag="ar1o", addr_space="Shared", name=f"ar1o_{c}")
            for mt in range(NDT):
                mm = pmm.tile([128, CS], F32, tag="mm")
                for j in range(HLOC):
                    nc.tensor.matmul(mm[:], p_wsb[:, j, mt * 128:(mt + 1) * 128],
                                     o_sb[:, j * S + c * CS: j * S + (c + 1) * CS],
                                     start=(j == 0), stop=(j == HLOC - 1))
                st = ps.tile([128, CS], COMM_DT, tag="cst")
                nc.scalar.activation(st[:], mm[:], ACTF.Identity,
                                     bias=pb_sb[:, mt:mt + 1], scale=1.0)
                nc.sync.dma_start(ar_in[mt * 128:(mt + 1) * 128, :], st[:])
            nc.gpsimd.collective_compute(
                "AllReduce", ALU.add, replica_groups=RG,
                ins=[ar_in[:].opt()], outs=[ar1_out[c][:].opt()])

        # MLP per chunk
        for c in range(CH):
            residual_add(c, ar1_out[c])
            negm, negm_w, rstd, sqv, rbs = ln_stats(c)
            xb = pbig.tile([128, NDT * CS], ATT_DT, tag="xbf")
            for dt in range(NDT):
                nc.vector.tensor_copy(xb[:, dt * CS:(dt + 1) * CS],
                                      hT[:, dt * S + c * CS: dt * S + (c + 1) * CS])
            f1a = pbig.tile([128, NF1 * CS], WGT_DT, tag="f1a")
            for mt in range(NF1):
                mm = pmm.tile([128, CS], F32, tag="mm")
                for kt in range(NDT):
                    nc.tensor.matmul(mm[:], f1_sb[:, kt, mt * 128:(mt + 1) * 128],
                                     xb[:, kt * CS:(kt + 1) * CS],
                                     start=(kt == 0), stop=False)
                nc.tensor.matmul(mm[:], f1cs_sb[:, mt * 128:(mt + 1) * 128],
                                 negm_w[:], start=False, stop=True)
                tmp = ps.tile([128, CS], F32, tag="t512")
                nc.vector.tensor_tensor(tmp[:], mm[:], rbs[:], op=ALU.mult)
                nc.scalar.activation(f1a[:, mt * CS:(mt + 1) * CS], tmp[:],
                                     ACTF.Gelu_apprx_tanh,
                                     bias=f1b_sb[:, mt:mt + 1], scale=1.0)
            ar_in = pd.tile([D, CS], COMM_DT, tag="ar2i")
            ar2_out_prev[c] = pd.tile([D, CS], COMM_DT, tag="ar2o", addr_space="Shared", name=f"ar2o_{c}")
            for mt in range(NDT):
                mm = pmm.tile([128, CS], F32, tag="mm")
                for kt in range(NF1):
                    nc.tensor.matmul(mm[:], f2_sb[:, kt, mt * 128:(mt + 1) * 128],
                                     f1a[:, kt * CS:(kt + 1) * CS],
                                     start=(kt == 0), stop=(kt == NF1 - 1))
                st = ps.tile([128, CS], COMM_DT, tag="cst")
                nc.scalar.activation(st[:], mm[:], ACTF.Identity,
                                     bias=f2b_sb[:, mt:mt + 1], scale=1.0)
                nc.sync.dma_start(ar_in[mt * 128:(mt + 1) * 128, :], st[:])
            nc.gpsimd.collective_compute(
                "AllReduce", ALU.add, replica_groups=RG,
                ins=[ar_in[:].opt()], outs=[ar2_out_prev[c][:].opt()])

    # ---- patch merger ----
    m0b_sb = pb.tile([128, NF1], F32, tag="m0b")
    nc.sync.dma_start(m0b_sb[:], t["m0b"][:].rearrange("(a p) -> p a", p=128))
    m2b_sb = pb.tile([128, OUT_DIM // 128], F32, tag="m2b")
    nc.sync.dma_start(m2b_sb[:], t["m2b"][:].rearrange("(a p) -> p a", p=128))

    ymc = [None] * CH
    for c in range(CH):
        residual_add(c, ar2_out_prev[c])
        negm, negm_w, rstd, sqv, rbs = ln_stats(c)
        nbp = pmm.tile([128, CS], F32, tag="mm")
        nc.tensor.matmul(nbp[:], ones_row[:], negm[:], start=True, stop=True)
        nbs = ps.tile([128, CS], F32, tag="t512")
        nc.scalar.copy(nbs[:], nbp[:])
        ymc[c] = pbig.tile([128, NDT * CS], ATT_DT, tag="xbf", name=f"ymc_{c}")
        for dt in range(NDT):
            sl = hT[:, dt * S + c * CS: dt * S + (c + 1) * CS]
            tmp = ps.tile([128, CS], F32, tag="t512")
            nc.vector.tensor_tensor(tmp[:], sl, nbs[:], op=ALU.add)
            nc.vector.tensor_tensor(ymc[c][:, dt * CS:(dt + 1) * CS],
                                    tmp[:], rbs[:], op=ALU.mult)

    ymg = [ymc[c][:].rearrange("p (a g f) -> p a f g", a=NDT, f=4)
           for c in range(CH)]
    m0a = pbig.tile([128, NF1 * G], WGT_DT, tag="m0a")
    for mt in range(NF1):
        mw = pqk.tile([128, 4 * NDT, 128], WGT_DT, tag="attT")
        nc.sync.dma_start(
            mw[:], t["m0T"][:, mt * 128:(mt + 1) * 128]
            .rearrange("(a p) m -> p a m", p=128))
        mm = pmm.tile([128, G], F32, tag="mm")
        for c in range(CH):
            for kk in range(4 * NDT):
                jj, dt = kk // NDT, kk % NDT
                nc.tensor.matmul(mm[:, c * GC:(c + 1) * GC],
                                 mw[:, kk, :], ymg[c][:, dt, jj, :],
                                 start=(kk == 0), stop=(kk == 4 * NDT - 1))
        nc.scalar.activation(m0a[:, mt * G:(mt + 1) * G], mm[:], ACTF.Gelu,
                             bias=m0b_sb[:, mt:mt + 1], scale=1.0)

    ar_in = pd.tile([OUT_DIM, G], COMM_DT, tag="ar3i")
    ar_out = pd.tile([OUT_DIM, G], COMM_DT, tag="ar3o", addr_space="Shared")
    for mt in range(OUT_DIM // 128):
        mw = pqk.tile([128, NF1, 128], WGT_DT, tag="m2w")
        nc.sync.dma_start(
            mw[:], t["m2T"][:, mt * 128:(mt + 1) * 128]
            .rearrange("(a p) m -> p a m", p=128))
        mm = pmm.tile([128, G], F32, tag="mm")
        for kt in range(NF1):
            nc.tensor.matmul(mm[:], mw[:, kt, :], m0a[:, kt * G:(kt + 1) * G],
                             start=(kt == 0), stop=(kt == NF1 - 1))
        st = ps.tile([128, G], COMM_DT, tag="cst")
        nc.scalar.activation(st[:], mm[:], ACTF.Identity,
                             bias=m2b_sb[:, mt:mt + 1], scale=1.0)
        nc.sync.dma_start(ar_in[mt * 128:(mt + 1) * 128, :], st[:])
    nc.gpsimd.collective_compute(
        "AllReduce", ALU.add, replica_groups=RG,
        ins=[ar_in[:].opt()], outs=[ar_out[:].opt()])
    for mt in range(OUT_DIM // 128):
        res = ps.tile([128, G], COMM_DT, tag="cst")
        nc.sync.dma_start(res[:], ar_out[mt * 128:(mt + 1) * 128, :])
        ob = ps.tile([128, G], F32, tag="t512")
        nc.vector.tensor_copy(ob[:], res[:])
        nc.sync.dma_start(t["out"][mt * 128:(mt + 1) * 128, :], ob[:])


# ------------------------------------------------------------------ host side

def _prep(inputs):
    f32 = np.float32
    pix = np.asarray(inputs["pixel_values"], f32)
    conv_w = np.asarray(inputs["conv_w"], f32)
    ln1_w, ln1_b = np.asarray(inputs["ln1_w"], f32), np.asarray(inputs["ln1_b"], f32)
    qkv_w, qkv_b = np.asarray(inputs["qkv_w"], f32), np.asarray(inputs["qkv_b"], f32)
    proj_w, proj_b = np.asarray(inputs["proj_w"], f32), np.asarray(inputs["proj_b"], f32)
    ln2_w, ln2_b = np.asarray(inputs["ln2_w"], f32), np.asarray(inputs["ln2_b"], f32)
    fc1_w, fc1_b = np.asarray(inputs["fc1_w"], f32), np.asarray(inputs["fc1_b"], f32)
    fc2_w, fc2_b = np.asarray(inputs["fc2_w"], f32), np.asarray(inputs["fc2_b"], f32)
    mln_w, mln_b = np.asarray(inputs["mln_w"], f32), np.asarray(inputs["mln_b"], f32)
    m0_w, m0_b = np.asarray(inputs["m0_w"], f32), np.asarray(inputs["m0_b"], f32)
    m2_w, m2_b = np.asarray(inputs["m2_w"], f32), np.asarray(inputs["m2_b"], f32)

    # im2col (pure indexing), zero-pad contraction dim 1176 -> 1280
    xr = pix[0].reshape(3, 1, 2, 32, 14, 32, 14)[:, 0]
    xT = np.zeros((D, S), f32)
    xT[:NPT] = xr.transpose(0, 1, 3, 5, 2, 4).reshape(NPT, S)
    convW = conv_w.reshape(D, NPT)

    # rope tables
    dd = HD // 2
    inv = 1.0 / (10000.0 ** (np.arange(0, dd, 2, dtype=f32) / dd))
    hpos = np.repeat(np.arange(32, dtype=f32), 32)
    wpos = np.tile(np.arange(32, dtype=f32), 32)
    rpe = np.concatenate([hpos[:, None] * inv, wpos[:, None] * inv], -1)
    emb = np.concatenate([rpe, rpe], -1)
    cosT, sinT = np.cos(emb).T.copy(), np.sin(emb).T.copy()
    rot = np.zeros((HD, HD), f32)
    for i in range(HD // 2):
        rot[2 * i + 1, 2 * i] = -1.0
        rot[2 * i, 2 * i + 1] = 1.0

    # fold LN affine into consumer weights; fold attn scale into Wq
    sc = HD ** -0.5
    Wqkv = qkv_w * ln1_w[:, None, :]
    bqkv = qkv_b + np.einsum("lcd,ld->lc", qkv_w, ln1_b)
    Wqkv[:, :D, :] *= sc
    bqkv[:, :D] *= sc
    W1 = fc1_w * ln2_w[:, None, :]
    b1 = fc1_b + np.einsum("lcd,ld->lc", fc1_w, ln2_b)
    mw4, mb4 = np.tile(mln_w, 4), np.tile(mln_b, 4)
    M0 = m0_w * mw4[None, :]
    b0 = m0_b + m0_w @ mb4

    has_qkv_b = bool(np.any(bqkv))
    in_maps = []
    for r in range(NCORE):
        hsel = slice(160 * r, 160 * r + 160)
        qkTl = np.zeros((DEPTH, D, QKC), f32)
        qkbl = np.zeros((DEPTH, QKC), f32)
        for j in range(HLOC):
            hs = slice(80 * (2 * r + j), 80 * (2 * r + j) + 80)
            qkTl[:, :, 128 * j:128 * j + 80] = Wqkv[:, hs, :].transpose(0, 2, 1)
            qkbl[:, 128 * j:128 * j + 80] = bqkv[:, hs]
            ks = slice(D + 80 * (2 * r + j), D + 80 * (2 * r + j) + 80)
            qkTl[:, :, 256 + 128 * j:256 + 128 * j + 80] = \
                Wqkv[:, ks, :].transpose(0, 2, 1)
            qkbl[:, 256 + 128 * j:256 + 128 * j + 80] = bqkv[:, ks]
        vsel = slice(2 * D + 160 * r, 2 * D + 160 * r + 160)
        vTl = np.ascontiguousarray(Wqkv[:, vsel, :].transpose(0, 2, 1))
        vbl = np.ascontiguousarray(bqkv[:, vsel])
        cvT = np.zeros((D, 2, 128), f32)
        wt = np.pad(convW[hsel].T, ((0, D - NPT), (0, 0)))
        cvT[:, 0, :128] = wt[:, :128]
        cvT[:, 1, :32] = wt[:, 128:160]
        m = {
            "xT": xT.astype(_NPA), "convT": cvT.astype(_NPA),
            "cosT": cosT.astype(_NPA), "sinT": sinT.astype(_NPA),
            "rotM": rot.astype(_NPA),
            "qkT": qkTl.astype(_NPW),
            "qkcs": qkTl.sum(axis=1).astype(_NPW),
            "vT": vTl.astype(_NPW), "vcs": vTl.sum(axis=1).astype(_NPW),
            "pT": np.ascontiguousarray(
                proj_w[:, :, hsel].transpose(0, 2, 1)
                .reshape(DEPTH, HLOC, HD, D)).astype(_NPA),
            "p_b": proj_b / NCORE,
            "f1T": np.ascontiguousarray(
                W1[:, 640 * r:640 * r + 640, :].transpose(0, 2, 1)).astype(_NPW),
            "f1cs": W1[:, 640 * r:640 * r + 640, :].sum(axis=2).astype(_NPW),
            "f1b": np.ascontiguousarray(b1[:, 640 * r:640 * r + 640]),
            "f2T": np.ascontiguousarray(
                fc2_w[:, :, 640 * r:640 * r + 640].transpose(0, 2, 1)).astype(_NPW),
            "f2b": fc2_b / NCORE,
            "m0T": np.ascontiguousarray(M0[640 * r:640 * r + 640, :].T).astype(_NPW),
            "m0b": np.ascontiguousarray(b0[640 * r:640 * r + 640]),
            "m2T": np.ascontiguousarray(m2_w[:, 640 * r:640 * r + 640].T).astype(_NPW),
            "m2b": m2_b / NCORE,
        }
        if has_qkv_b:
            m["qkb"] = qkbl
            m["vb"] = vbl
        in_maps.append(m)
    return in_maps, has_qkv_b


_CACHE = {}


def kernel(**inputs):
    in_maps, has_qkv_b = _prep(inputs)
    if has_qkv_b not in _CACHE:
        _CACHE[has_qkv_b] = _build(has_qkv_b)
    nc = _CACHE[has_qkv_b]
    res = bass_utils.run_bass_kernel_spmd(nc, in_maps,
                                          core_ids=list(range(NCORE)))
    o = np.asarray(res.results[0]["out"], np.float32)
    return np.ascontiguousarray(o.T)


# revision 17
# speedup vs baseline: 1.0678x; 1.0678x over previous
"""Trainium2 Bass kernel for an 8-block vision transformer (ArlowVisionTransformer).

Megatron-style tensor parallelism across the 8 NeuronCores of one chip: each
core owns 2 of 16 attention heads (qkv column-sharded), a 640-column slice of
fc1/m0 and the matching row slices of proj/fc2/m2; row-parallel partials are
AllReduced (bf16 bounce).

The residual stream is kept TRANSPOSED (features on SBUF partitions, tokens on
the free axis) so every linear layer consumes it directly with weight slices
stationary — no activation transposes.  LayerNorm uses "late normalization":
per-token stats come from ones-matmuls on TensorE, mean subtraction is folded
into each consumer matmul as a rank-1 accumulation, and the rstd scale is
applied to the (smaller) outputs.  LN affine params and the attention scale
are folded into weights on the host.  RoPE's rotate-half is an 80x80 matmul;
softmax denominators come from a ones-column appended to V.  Tokens run in 2
chunks so per-chunk AllReduces overlap the other chunk's compute.
"""

import sys

sys.path.insert(0, "/opt/trn_rl_repo")

import numpy as np
import ml_dtypes

import concourse.bass as bass
import concourse.bacc as bacc
import concourse.tile as tile
import concourse.mybir as mybir
from concourse import bass_utils

F32 = mybir.dt.float32
BF16 = mybir.dt.bfloat16
ALU = mybir.AluOpType
ACTF = mybir.ActivationFunctionType

DEPTH = 8
D = 1280
HEADS = 16
HD = 80
MLPD = 5120
OUT_DIM = 3584
S = 1024
NCORE = 8
HLOC = HEADS // NCORE
CH = 2
CS = S // CH
NDT = D // 128
NST = CS // 128
NF1 = 5
EPS = 1e-6
QKC = 512
FLOC = MLPD // NCORE
NPT = 1176
HB = 100                # per-head v block: 80 v + 16 zeros + 1 one + 3 pad
VW = 2 * HB
G = S // 4
GC = G // CH            # merged-token groups per chunk

COMM_DT = BF16
WGT_DT = BF16
ATT_DT = BF16

_NPW = ml_dtypes.bfloat16 if WGT_DT == BF16 else np.float32
_NPA = ml_dtypes.bfloat16 if ATT_DT == BF16 else np.float32


def _build(has_qkv_b):
    nc = bacc.Bacc("TRN2", target_bir_lowering=False, debug=False,
                   enable_asserts=False, num_devices=NCORE)

    def din(name, shape, dt=F32):
        return nc.dram_tensor(name, list(shape), dt, kind="ExternalInput")

    t = {}
    t["xT"] = din("xT", (D, S), ATT_DT)
    t["convT"] = din("convT", (D, NDT, 128), ATT_DT)
    t["cosT"] = din("cosT", (HD, S), ATT_DT)
    t["sinT"] = din("sinT", (HD, S), ATT_DT)
    t["rotM"] = din("rotM", (HD, HD), ATT_DT)
    t["qkT"] = din("qkT", (DEPTH, D, QKC), WGT_DT)
    t["qkcs"] = din("qkcs", (DEPTH, QKC), WGT_DT)
    t["vT"] = din("vT", (DEPTH, D, HLOC * HD), WGT_DT)
    t["vcs"] = din("vcs", (DEPTH, HLOC * HD), WGT_DT)
    t["pT"] = din("pT", (DEPTH, HLOC, HD, D), ATT_DT)
    t["p_b"] = din("p_b", (DEPTH, D))
    t["f1T"] = din("f1T", (DEPTH, D, FLOC), WGT_DT)
    t["f1cs"] = din("f1cs", (DEPTH, FLOC), WGT_DT)
    t["f1b"] = din("f1b", (DEPTH, FLOC))
    t["f2T"] = din("f2T", (DEPTH, FLOC, D), WGT_DT)
    t["f2b"] = din("f2b", (DEPTH, D))
    t["m0T"] = din("m0T", (MLPD, FLOC), WGT_DT)
    t["m0b"] = din("m0b", (FLOC,))
    t["m2T"] = din("m2T", (FLOC, OUT_DIM), WGT_DT)
    t["m2b"] = din("m2b", (OUT_DIM,))
    if has_qkv_b:
        t["qkb"] = din("qkb", (DEPTH, QKC))
        t["vb"] = din("vb", (DEPTH, HLOC * HD))
    t["out"] = nc.dram_tensor("out", [OUT_DIM, G], F32, kind="ExternalOutput")

    with tile.TileContext(nc) as tc:
        with (
            tc.tile_pool(name="const", bufs=1) as pc,
            tc.tile_pool(name="resid", bufs=1) as ph,
            tc.tile_pool(name="wgt", bufs=1) as pw,
            tc.tile_pool(name="wgt2", bufs=2) as pw2,
            tc.tile_pool(name="big", bufs=2) as pbig,
            tc.tile_pool(name="qkv", bufs=1) as pqk,
            tc.tile_pool(name="stg", bufs=3) as ps,
            tc.tile_pool(name="st2", bufs=2) as ps2,
            tc.tile_pool(name="bias", bufs=2) as pb,
            tc.tile_pool(name="pmm", bufs=3, space="PSUM") as pmm,
            tc.tile_pool(name="prow", bufs=1, space="PSUM") as prow,
            tc.tile_pool(name="pops", bufs=2, space="PSUM") as pops,
            tc.tile_pool(name="dram", bufs=2, space="DRAM") as pd,
        ):
            pools = dict(pc=pc, ph=ph, pw=pw, pw2=pw2, pbig=pbig, pqk=pqk, ps=ps,
                         ps2=ps2, pb=pb, pmm=pmm, prow=prow, pops=pops, pd=pd)
            _body(nc, tc, t, pools, has_qkv_b)
    nc.compile()
    return nc


def _body(nc, tc, t, p, has_qkv_b):
    pc, ph, pw, pw2, pbig, pqk = (p["pc"], p["ph"], p["pw"], p["pw2"],
                              p["pbig"], p["pqk"])
    ps, ps2, pb, pmm, prow, pops, pd = (p["ps"], p["ps2"], p["pb"], p["pmm"],
                                        p["prow"], p["pops"], p["pd"])
    RG = [list(range(NCORE))]

    # ---- constants ----
    ones_col = pc.tile([128, 1], F32)
    nc.gpsimd.memset(ones_col[:], 1.0)
    ones_row = pc.tile([1, 128], F32)
    nc.gpsimd.memset(ones_row[:], 1.0)
    cos_sb = pc.tile([HD, S], ATT_DT)
    nc.sync.dma_start(cos_sb[:], t["cosT"][:])
    sin_sb = pc.tile([HD, S], ATT_DT)
    nc.sync.dma_start(sin_sb[:], t["sinT"][:])
    rot_sb = pc.tile([HD, HD], ATT_DT)
    nc.sync.dma_start(rot_sb[:], t["rotM"][:])
    eps_sb = pc.tile([128, 1], F32)
    nc.gpsimd.memset(eps_sb[:], EPS)

    hT = ph.tile([128, NDT * S], F32)

    # ---- patchify (replicated, psum -> DRAM staging like before)
    xin = pbig.tile([128, NDT * CS], ATT_DT, tag="xbf", name="xin0")
    nc.sync.dma_start(xin[:].rearrange("p (a s) -> p a s", a=NDT),
                      t["xT"][:, 0:CS].rearrange("(a p) s -> p a s", p=128))
    xin2 = pbig.tile([128, NDT * CS], ATT_DT, tag="xbf", name="xin1")
    nc.sync.dma_start(xin2[:].rearrange("p (a s) -> p a s", a=NDT),
                      t["xT"][:, CS:S].rearrange("(a p) s -> p a s", p=128))
    xins = [xin, xin2]
    for mt in range(NDT):
        cwm = ps.tile([128, NDT, 128], ATT_DT, tag="t512")
        nc.sync.dma_start(
            cwm[:], t["convT"][:, mt, :].rearrange("(a p) x -> p a x", p=128))
        for c in range(CH):
            acc = pmm.tile([128, CS], F32, tag="mm")
            for kt in range(NDT):
                nc.tensor.matmul(acc[:], cwm[:, kt, :],
                                 xins[c][:, kt * CS:(kt + 1) * CS],
                                 start=(kt == 0), stop=(kt == NDT - 1))
            nc.scalar.copy(hT[:, mt * S + c * CS: mt * S + (c + 1) * CS], acc[:])

    # ---- helpers ----
    ones_bf = pc.tile([128, 1], BF16)
    nc.gpsimd.memset(ones_bf[:], 1.0)

    def ln_stats(xb, want_rcol=False):
        # stats of the bf16 copy xb (what the matmuls actually consume)
        sums = prow.tile([1, CS], F32, tag="srow")
        sq = prow.tile([1, CS], F32, tag="sqrow")
        for dt in range(NDT):
            sl = xb[:, dt * CS:(dt + 1) * CS]
            x2 = ps.tile([128, CS], ATT_DT, tag="x2")
            nc.scalar.square(x2[:], sl)
            nc.tensor.matmul(sums[:], ones_bf[:], sl,
                             start=(dt == 0), stop=(dt == NDT - 1))
            nc.tensor.matmul(sq[:], ones_bf[:], x2[:],
                             start=(dt == 0), stop=(dt == NDT - 1))
        negm = ps2.tile([1, CS], F32, tag="negm")
        nc.vector.tensor_scalar_mul(negm[:], sums[:], -1.0 / D)
        msq = ps2.tile([1, CS], F32, tag="rowA")
        nc.vector.tensor_tensor(msq[:], negm[:], negm[:], op=ALU.mult)
        var = ps2.tile([1, CS], F32, tag="rowB")
        nc.vector.scalar_tensor_tensor(var[:], sq[:], 1.0 / D, msq[:],
                                       op0=ALU.mult, op1=ALU.subtract)
        sqv = ps2.tile([1, CS], F32, tag="rowA")
        nc.scalar.activation(sqv[:], var[:], ACTF.Sqrt, bias=eps_sb[0:1, :], scale=1.0)
        rstd = ps2.tile([1, CS], F32, tag="rowA")
        nc.vector.reciprocal(rstd[:], sqv[:])
        rbp = pmm.tile([128, CS], F32, tag="mm")
        nc.tensor.matmul(rbp[:], ones_row[:], rstd[:], start=True, stop=True)
        rbs = ps2.tile([128, CS], F32, tag="rbs")
        nc.scalar.copy(rbs[:], rbp[:])
        rcol = None
        if want_rcol:
            rtmp1 = pd.tile([CS], F32, tag="rtmp1")
            nc.sync.dma_start(rtmp1[:], rstd[0:1, :])
            rcol = ps.tile([128, NST], F32, tag="rcol")
            nc.sync.dma_start(
                rcol[:], rtmp1[:].rearrange("(a p) -> p a", p=128))
        negm_w = negm
        if WGT_DT != F32:
            negm_w = ps2.tile([1, CS], WGT_DT, tag="negmw")
            nc.vector.tensor_copy(negm_w[:], negm[:])
        return negm, negm_w, sqv, rbs, rcol

    def residual_add(c, ar_dram):
        for dt in range(NDT):
            ars = ps.tile([128, CS], COMM_DT, tag="cst")
            nc.sync.dma_start(ars[:], ar_dram[dt * 128:(dt + 1) * 128, :])
            sl = hT[:, dt * S + c * CS: dt * S + (c + 1) * CS]
            nc.gpsimd.tensor_tensor(sl, ars[:], sl, op=ALU.add)

    ar2_out_prev = [None, None]

    # ---- transformer blocks ----
    def load_qkw(l):
        w = {}
        w["qk"] = pw2.tile([128, NDT, QKC], WGT_DT, tag="qkT", name=f"qkw{l}")
        nc.sync.dma_start(w["qk"][:],
                          t["qkT"][l].rearrange("(a p) m -> p a m", p=128))
        w["qkcs"] = pb.tile([1, QKC], WGT_DT, tag="qkcs", name=f"qkcs{l}")
        nc.sync.dma_start(w["qkcs"][:], t["qkcs"][l:l + 1, :])
        w["v"] = pw2.tile([128, NDT, HLOC * HD], WGT_DT, tag="vT",
                          name=f"vw{l}")
        nc.sync.dma_start(w["v"][:],
                          t["vT"][l].rearrange("(a p) m -> p a m", p=128))
        w["vcs"] = pb.tile([1, HLOC * HD], WGT_DT, tag="vcs", name=f"vcs{l}")
        nc.sync.dma_start(w["vcs"][:], t["vcs"][l:l + 1, :])
        if has_qkv_b:
            w["qkb"] = pb.tile([1, QKC], F32, tag="qkb", name=f"qkb{l}")
            nc.sync.dma_start(w["qkb"][:], t["qkb"][l:l + 1, :])
            w["vb"] = pb.tile([1, HLOC * HD], F32, tag="vb", name=f"vb{l}")
            nc.sync.dma_start(w["vb"][:], t["vb"][l:l + 1, :])
        return w

    wq = load_qkw(0)
    for l in range(DEPTH):
        qk_sb, qkcs_sb = wq["qk"], wq["qkcs"]
        v_wsb, vcs_sb = wq["v"], wq["vcs"]
        if has_qkv_b:
            qkb_sb, vb_sb = wq["qkb"], wq["vb"]
        p_wsb = pw.tile([HD, HLOC, D], ATT_DT, tag="pT")
        nc.sync.dma_start(p_wsb[:], t["pT"][l].rearrange("j d m -> d j m"))
        pb_sb = pb.tile([128, NDT], F32, tag="pb")
        nc.sync.dma_start(pb_sb[:], t["p_b"][l].rearrange("(a p) -> p a", p=128))
        f1_sb = pw.tile([128, NDT, FLOC], WGT_DT, tag="f1T")
        nc.sync.dma_start(f1_sb[:],
                          t["f1T"][l].rearrange("(a p) m -> p a m", p=128))
        f1cs_sb = pb.tile([1, FLOC], WGT_DT, tag="f1cs")
        nc.sync.dma_start(f1cs_sb[:], t["f1cs"][l:l + 1, :])
        f1b_sb = pb.tile([128, NF1], F32, tag="f1b")
        nc.sync.dma_start(f1b_sb[:], t["f1b"][l].rearrange("(a p) -> p a", p=128))
        f2_sb = pw.tile([128, NF1, D], WGT_DT, tag="f2T")
        nc.sync.dma_start(f2_sb[:],
                          t["f2T"][l].rearrange("(a p) m -> p a m", p=128))
        f2b_sb = pb.tile([128, NDT], F32, tag="f2b")
        nc.sync.dma_start(f2b_sb[:], t["f2b"][l].rearrange("(a p) -> p a", p=128))
        q_sb = pqk.tile([HD, HLOC * S], ATT_DT, tag="qsb")
        k_sb = pqk.tile([HD, HLOC * S], ATT_DT, tag="ksb")
        v_sb = pqk.tile([128, (S // 128) * VW], ATT_DT, tag="vsb")
        xbf = [None] * CH

        for c in range(CH):
            if ar2_out_prev[c] is not None:
                residual_add(c, ar2_out_prev[c])
                ar2_out_prev[c] = None

            xb = pbig.tile([128, NDT * CS], ATT_DT, tag="xbf")
            for dt in range(NDT):
                nc.vector.tensor_copy(xb[:, dt * CS:(dt + 1) * CS],
                                      hT[:, dt * S + c * CS: dt * S + (c + 1) * CS])
            xbf[c] = xb
            negm, negm_w, sqv, rbs, rcol = ln_stats(xb, want_rcol=True)

            # q,k: psum = W^T x - colsum (x) mean [+ b (x) sqrtvar]
            for mt in range(4):
                mm = pmm.tile([128, CS], F32, tag="mm")
                for kt in range(NDT):
                    nc.tensor.matmul(mm[:], qk_sb[:, kt, mt * 128:(mt + 1) * 128],
                                     xb[:, kt * CS:(kt + 1) * CS],
                                     start=(kt == 0), stop=False)
                nc.tensor.matmul(mm[:], qkcs_sb[:, mt * 128:(mt + 1) * 128],
                                 negm_w[:], start=False, stop=not has_qkv_b)
                if has_qkv_b:
                    nc.tensor.matmul(mm[:], qkb_sb[:, mt * 128:(mt + 1) * 128],
                                     sqv[:], start=False, stop=True)
                t1 = ps.tile([HD, CS], ATT_DT, tag="tro")
                nc.vector.tensor_tensor(t1[:], mm[:HD, :], rbs[:HD, :],
                                        op=ALU.mult)
                rot = pmm.tile([HD, CS], F32, tag="mm")
                nc.tensor.matmul(rot[:], rot_sb[:], t1[:], start=True, stop=True)
                j, is_k = mt % 2, mt // 2
                dst = (k_sb if is_k else q_sb)[:, j * S + c * CS: j * S + (c + 1) * CS]
                t2 = ps.tile([HD, CS], ATT_DT, tag="tro")
                nc.vector.tensor_tensor(t2[:], t1[:],
                                        cos_sb[:, c * CS:(c + 1) * CS], op=ALU.mult)
                t3 = ps.tile([HD, CS], ATT_DT, tag="tro")
                nc.vector.tensor_tensor(t3[:], rot[:],
                                        sin_sb[:, c * CS:(c + 1) * CS], op=ALU.mult)
                nc.vector.tensor_tensor(dst, t2[:], t3[:], op=ALU.add)

            # v (normal layout) scaled by rstd per token
            for st in range(NST):
                mm = pmm.tile([128, HLOC * HD], F32, tag="mm")
                for kt in range(NDT):
                    nc.tensor.matmul(
                        mm[:], xb[:, kt * CS + st * 128: kt * CS + (st + 1) * 128],
                        v_wsb[:, kt, :], start=(kt == 0), stop=False)
                nc.tensor.matmul(mm[:], negm_w[0:1, st * 128:(st + 1) * 128],
                                 vcs_sb[:], start=False, stop=not has_qkv_b)
                if has_qkv_b:
                    nc.tensor.matmul(mm[:], sqv[0:1, st * 128:(st + 1) * 128],
                                     vb_sb[:], start=False, stop=True)
                tt = c * NST + st
                for j in range(HLOC):
                    base = tt * VW + j * HB
                    nc.vector.tensor_scalar_mul(v_sb[:, base:base + HD],
                                                mm[:, j * HD:(j + 1) * HD],
                                                rcol[:, st:st + 1])
                    nc.gpsimd.memset(v_sb[:, base + HD:base + 96], 0.0)
                    nc.gpsimd.memset(v_sb[:, base + 96:base + 97], 1.0)

        if l + 1 < DEPTH:
            wq = load_qkw(l + 1)

        # attention
        o_sb = pqk.tile([HD, HLOC * S], ATT_DT, tag="osb")
        for j in range(HLOC):
            for c in range(CH):
                att = pqk.tile([128, (S // 128) * CS], ATT_DT, tag="attT")
                for tt in range(S // 128):
                    mm = pmm.tile([128, CS], F32, tag="mm")
                    nc.tensor.matmul(
                        mm[:], k_sb[:, j * S + tt * 128:j * S + (tt + 1) * 128],
                        q_sb[:, j * S + c * CS:j * S + (c + 1) * CS],
                        start=True, stop=True)
                    nc.scalar.activation(att[:, tt * CS:(tt + 1) * CS], mm[:],
                                         ACTF.Exp)
                ops = pops.tile([97, CS], F32, tag="ops")
                for tt in range(S // 128):
                    base = tt * VW + j * HB
                    nc.tensor.matmul(ops[:], v_sb[:, base:base + 97],
                                     att[:, tt * CS:(tt + 1) * CS],
                                     start=(tt == 0), stop=(tt == S // 128 - 1))
                rec = ps2.tile([1, CS], F32, tag="rowB")
                nc.vector.reciprocal(rec[:], ops[96:97, :])
                rb2p = pmm.tile([HD, CS], F32, tag="mm")
                nc.tensor.matmul(rb2p[:], ones_row[0:1, :HD], rec[:],
                                 start=True, stop=True)
                rb2 = ps2.tile([HD, CS], F32, tag="rb2")
                nc.scalar.copy(rb2[:], rb2p[:])
                nc.vector.tensor_tensor(
                    o_sb[:, j * S + c * CS: j * S + (c + 1) * CS],
                    ops[:HD, :], rb2[:], op=ALU.mult)

        # proj partials -> AllReduce per chunk
        ar1_out = [None] * CH
        for c in range(CH):
            ar_in = pd.tile([D, CS], COMM_DT, tag="ar1i")
            ar1_out[c] = pd.tile([D, CS], COMM_DT, tag="ar1o", addr_space="Shared", name=f"ar1o_{c}")
            for mt in range(NDT):
                mm = pmm.tile([128, CS], F32, tag="mm")
                for j in range(HLOC):
                    nc.tensor.matmul(mm[:], p_wsb[:, j, mt * 128:(mt + 1) * 128],
                                     o_sb[:, j * S + c * CS: j * S + (c + 1) * CS],
                                     start=(j == 0), stop=(j == HLOC - 1))
                st = ps.tile([128, CS], COMM_DT, tag="cst")
                nc.scalar.activation(st[:], mm[:], ACTF.Identity,
                                     bias=pb_sb[:, mt:mt + 1], scale=1.0)
                nc.sync.dma_start(ar_in[mt * 128:(mt + 1) * 128, :], st[:])
            nc.gpsimd.collective_compute(
                "AllReduce", ALU.add, replica_groups=RG,
                ins=[ar_in[:].opt()], outs=[ar1_out[c][:].opt()])

        # MLP per chunk
        for c in range(CH):
            residual_add(c, ar1_out[c])
            xb = pbig.tile([128, NDT * CS], ATT_DT, tag="xbf")
            for dt in range(NDT):
                nc.vector.tensor_copy(xb[:, dt * CS:(dt + 1) * CS],
                                      hT[:, dt * S + c * CS: dt * S + (c + 1) * CS])
            negm, negm_w, sqv, rbs, _ = ln_stats(xb)
            f1a = pbig.tile([128, NF1 * CS], WGT_DT, tag="f1a")
            for mt in range(NF1):
                mm = pmm.tile([128, CS], F32, tag="mm")
                for kt in range(NDT):
                    nc.tensor.matmul(mm[:], f1_sb[:, kt, mt * 128:(mt + 1) * 128],
                                     xb[:, kt * CS:(kt + 1) * CS],
                                     start=(kt == 0), stop=False)
                nc.tensor.matmul(mm[:], f1cs_sb[:, mt * 128:(mt + 1) * 128],
                                 negm_w[:], start=False, stop=True)
                tmp = ps.tile([128, CS], F32, tag="t512")
                nc.vector.tensor_tensor(tmp[:], mm[:], rbs[:], op=ALU.mult)
                nc.scalar.activation(f1a[:, mt * CS:(mt + 1) * CS], tmp[:],
                                     ACTF.Gelu_apprx_tanh,
                                     bias=f1b_sb[:, mt:mt + 1], scale=1.0)
            ar_in = pd.tile([D, CS], COMM_DT, tag="ar2i")
            ar2_out_prev[c] = pd.tile([D, CS], COMM_DT, tag="ar2o", addr_space="Shared", name=f"ar2o_{c}")
            for mt in range(NDT):
                mm = pmm.tile([128, CS], F32, tag="mm")
                for kt in range(NF1):
                    nc.tensor.matmul(mm[:], f2_sb[:, kt, mt * 128:(mt + 1) * 128],
                                     f1a[:, kt * CS:(kt + 1) * CS],
                                     start=(kt == 0), stop=(kt == NF1 - 1))
                st = ps.tile([128, CS], COMM_DT, tag="cst")
                nc.scalar.activation(st[:], mm[:], ACTF.Identity,
                                     bias=f2b_sb[:, mt:mt + 1], scale=1.0)
                nc.sync.dma_start(ar_in[mt * 128:(mt + 1) * 128, :], st[:])
            nc.gpsimd.collective_compute(
                "AllReduce", ALU.add, replica_groups=RG,
                ins=[ar_in[:].opt()], outs=[ar2_out_prev[c][:].opt()])

    # ---- patch merger ----
    m0b_sb = pb.tile([128, NF1], F32, tag="m0b")
    nc.sync.dma_start(m0b_sb[:], t["m0b"][:].rearrange("(a p) -> p a", p=128))
    m2b_sb = pb.tile([128, OUT_DIM // 128], F32, tag="m2b")
    nc.sync.dma_start(m2b_sb[:], t["m2b"][:].rearrange("(a p) -> p a", p=128))

    ymc = [None] * CH
    for c in range(CH):
        residual_add(c, ar2_out_prev[c])
        xb = pbig.tile([128, NDT * CS], ATT_DT, tag="xbf", name=f"xbm_{c}")
        for dt in range(NDT):
            nc.vector.tensor_copy(xb[:, dt * CS:(dt + 1) * CS],
                                  hT[:, dt * S + c * CS: dt * S + (c + 1) * CS])
        negm, negm_w, sqv, rbs, _ = ln_stats(xb)
        nbp = pmm.tile([128, CS], F32, tag="mm")
        nc.tensor.matmul(nbp[:], ones_row[:], negm[:], start=True, stop=True)
        nbs = ps2.tile([128, CS], F32, tag="rb2")
        nc.scalar.copy(nbs[:], nbp[:])
        ymc[c] = xb
        for dt in range(NDT):
            sl = xb[:, dt * CS:(dt + 1) * CS]
            nc.vector.tensor_tensor(sl, sl, nbs[:], op=ALU.add)
            nc.vector.tensor_tensor(sl, sl, rbs[:], op=ALU.mult)

    ymg = [ymc[c][:].rearrange("p (a g f) -> p a f g", a=NDT, f=4)
           for c in range(CH)]
    m0a = pbig.tile([128, NF1 * G], WGT_DT, tag="m0a")
    for mt in range(NF1):
        mw = pqk.tile([128, 4 * NDT, 128], WGT_DT, tag="attT")
        nc.sync.dma_start(
            mw[:], t["m0T"][:, mt * 128:(mt + 1) * 128]
            .rearrange("(a p) m -> p a m", p=128))
        mm = pmm.tile([128, G], F32, tag="mm")
        for c in range(CH):
            for kk in range(4 * NDT):
                jj, dt = kk // NDT, kk % NDT
                nc.tensor.matmul(mm[:, c * GC:(c + 1) * GC],
                                 mw[:, kk, :], ymg[c][:, dt, jj, :],
                                 start=(kk == 0), stop=(kk == 4 * NDT - 1))
        nc.scalar.activation(m0a[:, mt * G:(mt + 1) * G], mm[:], ACTF.Gelu,
                             bias=m0b_sb[:, mt:mt + 1], scale=1.0)

    ar_in = pd.tile([OUT_DIM, G], COMM_DT, tag="ar3i")
    ar_out = pd.tile([OUT_DIM, G], COMM_DT, tag="ar3o", addr_space="Shared")
    for mt in range(OUT_DIM // 128):
        mw = pqk.tile([128, NF1, 128], WGT_DT, tag="m2w")
        nc.sync.dma_start(
            mw[:], t["m2T"][:, mt * 128:(mt + 1) * 128]
            .rearrange("(a p) m -> p a m", p=128))
        mm = pmm.tile([128, G], F32, tag="mm")
        for kt in range(NF1):
            nc.tensor.matmul(mm[:], mw[:, kt, :], m0a[:, kt * G:(kt + 1) * G],
                             start=(kt == 0), stop=(kt == NF1 - 1))
        st = ps.tile([128, G], COMM_DT, tag="cst")
        nc.scalar.activation(st[:], mm[:], ACTF.Identity,
                             bias=m2b_sb[:, mt:mt + 1], scale=1.0)
        nc.sync.dma_start(ar_in[mt * 128:(mt + 1) * 128, :], st[:])
    nc.gpsimd.collective_compute(
        "AllReduce", ALU.add, replica_groups=RG,
        ins=[ar_in[:].opt()], outs=[ar_out[:].opt()])
    for mt in range(OUT_DIM // 128):
        res = ps.tile([128, G], COMM_DT, tag="cst")
        nc.sync.dma_start(res[:], ar_out[mt * 128:(mt + 1) * 128, :])
        ob = ps.tile([128, G], F32, tag="t512")
        nc.vector.tensor_copy(ob[:], res[:])
        nc.sync.dma_start(t["out"][mt * 128:(mt + 1) * 128, :], ob[:])


# ------------------------------------------------------------------ host side

def _prep(inputs):
    f32 = np.float32
    pix = np.asarray(inputs["pixel_values"], f32)
    conv_w = np.asarray(inputs["conv_w"], f32)
    ln1_w, ln1_b = np.asarray(inputs["ln1_w"], f32), np.asarray(inputs["ln1_b"], f32)
    qkv_w, qkv_b = np.asarray(inputs["qkv_w"], f32), np.asarray(inputs["qkv_b"], f32)
    proj_w, proj_b = np.asarray(inputs["proj_w"], f32), np.asarray(inputs["proj_b"], f32)
    ln2_w, ln2_b = np.asarray(inputs["ln2_w"], f32), np.asarray(inputs["ln2_b"], f32)
    fc1_w, fc1_b = np.asarray(inputs["fc1_w"], f32), np.asarray(inputs["fc1_b"], f32)
    fc2_w, fc2_b = np.asarray(inputs["fc2_w"], f32), np.asarray(inputs["fc2_b"], f32)
    mln_w, mln_b = np.asarray(inputs["mln_w"], f32), np.asarray(inputs["mln_b"], f32)
    m0_w, m0_b = np.asarray(inputs["m0_w"], f32), np.asarray(inputs["m0_b"], f32)
    m2_w, m2_b = np.asarray(inputs["m2_w"], f32), np.asarray(inputs["m2_b"], f32)

    # im2col (pure indexing), zero-pad contraction dim 1176 -> 1280
    xr = pix[0].reshape(3, 1, 2, 32, 14, 32, 14)[:, 0]
    xT = np.zeros((D, S), f32)
    xT[:NPT] = xr.transpose(0, 1, 3, 5, 2, 4).reshape(NPT, S)
    convW = conv_w.reshape(D, NPT)

    # rope tables
    dd = HD // 2
    inv = 1.0 / (10000.0 ** (np.arange(0, dd, 2, dtype=f32) / dd))
    hpos = np.repeat(np.arange(32, dtype=f32), 32)
    wpos = np.tile(np.arange(32, dtype=f32), 32)
    rpe = np.concatenate([hpos[:, None] * inv, wpos[:, None] * inv], -1)
    emb = np.concatenate([rpe, rpe], -1)
    cosT, sinT = np.cos(emb).T.copy(), np.sin(emb).T.copy()
    rot = np.zeros((HD, HD), f32)
    for i in range(HD // 2):
        rot[2 * i + 1, 2 * i] = -1.0
        rot[2 * i, 2 * i + 1] = 1.0

    # fold LN affine into consumer weights; fold attn scale into Wq
    sc = HD ** -0.5
    Wqkv = qkv_w * ln1_w[:, None, :]
    bqkv = qkv_b + np.einsum("lcd,ld->lc", qkv_w, ln1_b)
    Wqkv[:, :D, :] *= sc
    bqkv[:, :D] *= sc
    W1 = fc1_w * ln2_w[:, None, :]
    b1 = fc1_b + np.einsum("lcd,ld->lc", fc1_w, ln2_b)
    mw4, mb4 = np.tile(mln_w, 4), np.tile(mln_b, 4)
    M0 = m0_w * mw4[None, :]
    b0 = m0_b + m0_w @ mb4

    has_qkv_b = bool(np.any(bqkv))
    in_maps = []
    for r in range(NCORE):
        hsel = slice(160 * r, 160 * r + 160)
        qkTl = np.zeros((DEPTH, D, QKC), f32)
        qkbl = np.zeros((DEPTH, QKC), f32)
        for j in range(HLOC):
            hs = slice(80 * (2 * r + j), 80 * (2 * r + j) + 80)
            qkTl[:, :, 128 * j:128 * j + 80] = Wqkv[:, hs, :].transpose(0, 2, 1)
            qkbl[:, 128 * j:128 * j + 80] = bqkv[:, hs]
            ks = slice(D + 80 * (2 * r + j), D + 80 * (2 * r + j) + 80)
            qkTl[:, :, 256 + 128 * j:256 + 128 * j + 80] = \
                Wqkv[:, ks, :].transpose(0, 2, 1)
            qkbl[:, 256 + 128 * j:256 + 128 * j + 80] = bqkv[:, ks]
        vsel = slice(2 * D + 160 * r, 2 * D + 160 * r + 160)
        vTl = np.ascontiguousarray(Wqkv[:, vsel, :].transpose(0, 2, 1))
        vbl = np.ascontiguousarray(bqkv[:, vsel])
        cvT = np.pad(convW.T, ((0, D - NPT), (0, 0))).reshape(D, NDT, 128)
        m = {
            "xT": xT.astype(_NPA), "convT": cvT.astype(_NPA),
            "cosT": cosT.astype(_NPA), "sinT": sinT.astype(_NPA),
            "rotM": rot.astype(_NPA),
            "qkT": qkTl.astype(_NPW),
            "qkcs": qkTl.sum(axis=1).astype(_NPW),
            "vT": vTl.astype(_NPW), "vcs": vTl.sum(axis=1).astype(_NPW),
            "pT": np.ascontiguousarray(
                proj_w[:, :, hsel].transpose(0, 2, 1)
                .reshape(DEPTH, HLOC, HD, D)).astype(_NPA),
            "p_b": proj_b / NCORE,
            "f1T": np.ascontiguousarray(
                W1[:, 640 * r:640 * r + 640, :].transpose(0, 2, 1)).astype(_NPW),
            "f1cs": W1[:, 640 * r:640 * r + 640, :].sum(axis=2).astype(_NPW),
            "f1b": np.ascontiguousarray(b1[:, 640 * r:640 * r + 640]),
            "f2T": np.ascontiguousarray(
                fc2_w[:, :, 640 * r:640 * r + 640].transpose(0, 2, 1)).astype(_NPW),
            "f2b": fc2_b / NCORE,
            "m0T": np.ascontiguousarray(M0[640 * r:640 * r + 640, :].T).astype(_NPW),
            "m0b": np.ascontiguousarray(b0[640 * r:640 * r + 640]),
            "m2T": np.ascontiguousarray(m2_w[:, 640 * r:640 * r + 640].T).astype(_NPW),
            "m2b": m2_b / NCORE,
        }
        if has_qkv_b:
            m["qkb"] = qkbl
            m["vb"] = vbl
        in_maps.append(m)
    return in_maps, has_qkv_b


_CACHE = {}


def kernel(**inputs):
    in_maps, has_qkv_b = _prep(inputs)
    if has_qkv_b not in _CACHE:
        _CACHE[has_qkv_b] = _build(has_qkv_b)
    nc = _CACHE[has_qkv_b]
    res = bass_utils.run_bass_kernel_spmd(nc, in_maps,
                                          core_ids=list(range(NCORE)))
    o = np.asarray(res.results[0]["out"], np.float32)
    return np.ascontiguousarray(o.T)
